# revision 1
# baseline (speedup 1.0000x reference)
"""Trainium2 Bass kernel for the ButterflyMlp problem.

Computes log_softmax(L3(relu(L2(relu(L1(x)))))) where each Li is a masked
linear layer (butterfly sparsity: global column stripes + a diagonal band),
batch 65536, data-parallel over 8 NeuronCores (8192 rows/core).

Strategy (per core, feature-major throughout):
  - Masks are pre-applied to weights on host. Layer-1 exploits the butterfly
    structure: the stripe columns (mask true for every output row) form a
    dense [|S|, 784] GEMM shared by all outputs, and the per-output-block
    band adds one narrow [|R_j|<=128, 112] GEMM per 112-row output block.
    This cuts layer-1 matmul passes from 49 to 21 per batch chunk.
  - All GEMMs run in float32r (fp32 high-pass mode: ~1.6e-4 relative error,
    ~1.8 cycles/row vs 4 for plain fp32).
  - Bulk data moves through SWDGE (nc.gpsimd.dma_start), which stripes one
    transfer across all 16 SDMA engines (~340 GB/s); the HWDGE rings
    (sync/scalar) are single-engine (~19 GB/s) and carry only small
    tensors and result stores. x rows are pre-gathered AND pre-packed on
    host so each 1024-column superchunk needs exactly one stripe DMA and
    one band DMA (band blocks padded to a common partition count).
  - ReLU+bias fuses into the PSUM->SBUF eviction, alternating ScalarE and
    VectorE.
  - log_softmax stays feature-major, batched per superchunk: exp (ACT) ->
    partition_all_reduce (GpSimd) -> ln (ACT) -> subtract (DVE). No max
    subtraction needed (logits are O(1); exp cannot overflow).
  - Output is [10, 8192] per core; host transposes and concatenates.
"""
import sys
sys.path.insert(0, "/opt/trn_rl_repo")
import numpy as np

import concourse.bass as bass
import concourse.bacc as bacc
import concourse.mybir as mybir
import concourse.tile as tile
import concourse.bass_isa as bass_isa
from concourse import bass_utils

import os
F32 = mybir.dt.float32
_MM_DT_NAME = os.environ.get("BUTTERFLY_MM_DT", "float16")
F32R = getattr(mybir.dt, _MM_DT_NAME)      # matmul operand dtype
_MM_NP = {"float32r": np.float32, "float16": np.float16,
          "bfloat16": None}[_MM_DT_NAME]
AF = mybir.ActivationFunctionType
ALU = mybir.AluOpType

# All activation functions this kernel uses live together in the
# natural_log_exp_and_others table set, but the greedy per-function set
# chooser picks exp_and_others for Exp and natural_log* for Ln, reloading
# ACT tables twice per chunk (~1.3us each). Restrict every other set's
# advertised contents so the chooser lands on the one set that covers
# everything and emits a single load. Set ids stay valid: the dict keys
# and order are unchanged.
_PIN_SET = "natural_log_exp_and_others"
_orig_gat = bacc.get_activation_tables


def _pinned_gat(arch):
    tabs = _orig_gat(arch)
    need = {AF.Relu, AF.Identity, AF.Exp, AF.Ln, AF.Copy}
    if _PIN_SET in tabs and need <= tabs[_PIN_SET]:
        for name in tabs:
            if name != _PIN_SET:
                tabs[name] = tabs[name] - need
    return tabs


bacc.get_activation_tables = _pinned_gat

N_CORES = 8
NB = 512          # batch columns per matmul (one PSUM bank of fp32)
SC = 1024         # batch columns per DMA superchunk / epilogue batch
OT = 112          # layer-1 output block width (784/7; band window fits 128)


def _decompose_mask1(mask1):
    """Split the butterfly mask into stripe columns S (true for every row)
    and per-output-block residual columns R_j."""
    D_out, D_in = mask1.shape
    S = np.where(mask1.all(axis=0))[0]
    n_blk = (D_out + OT - 1) // OT
    stripe_set = np.zeros(D_in, dtype=bool)
    stripe_set[S] = True
    R_list = []
    for j in range(n_blk):
        blk = mask1[j * OT:(j + 1) * OT]
        cols = np.where(blk.any(axis=0) & ~stripe_set)[0]
        assert len(cols) <= 128, f"band block {j} has {len(cols)} cols"
        R_list.append(cols)
    return S, R_list


def _build_program(meta):
    nS, R_lens = meta["nS"], meta["R_lens"]
    P_pad = meta["P_pad"]
    Bc = meta["Bc"]
    D1, H, C = meta["D1"], meta["H"], meta["C"]
    n_blk = len(R_lens)
    n_sc = (nS + 127) // 128              # stripe K-chunks
    sc_w = -(-nS // n_sc)                 # stripe chunk width (padded)
    n_kc2 = D1 // OT                      # layer-2 K chunks (= n_blk)
    n_sup = Bc // SC                      # DMA superchunks
    n_half = SC // NB                     # matmul chunks per superchunk

    nc = bacc.Bacc("TRN2", target_bir_lowering=False, debug=False,
                   enable_asserts=False, num_devices=N_CORES)

    xs_d = nc.dram_tensor("xs", [sc_w, n_sc * Bc], F32R,
                          kind="ExternalInput").ap()
    xb_d = nc.dram_tensor("xb", [P_pad, n_blk * Bc], F32R,
                          kind="ExternalInput").ap()
    ws_d = nc.dram_tensor("ws", [sc_w, n_sc * D1], F32R,
                          kind="ExternalInput").ap()
    wb_d = nc.dram_tensor("wb", [P_pad, n_blk * OT], F32R,
                          kind="ExternalInput").ap()
    w2_d = nc.dram_tensor("w2", [OT, n_kc2 * H], F32R, kind="ExternalInput").ap()
    w3_d = nc.dram_tensor("w3", [H, C], F32R, kind="ExternalInput").ap()
    b1_d = nc.dram_tensor("b1", [OT, n_blk], F32, kind="ExternalInput").ap()
    b2_d = nc.dram_tensor("b2", [H, 1], F32, kind="ExternalInput").ap()
    b3_d = nc.dram_tensor("b3", [C, 1], F32, kind="ExternalInput").ap()
    ones_d = nc.dram_tensor("ones", [C, C], F32R, kind="ExternalInput").ap()
    out_d = nc.dram_tensor("out", [C, Bc], F32, kind="ExternalOutput").ap()


    with tile.TileContext(nc) as tc:
        with tc.tile_pool(name="wp", bufs=1) as wp, \
             tc.tile_pool(name="xp", bufs=3) as xp, \
             tc.tile_pool(name="hp", bufs=2) as hp, \
             tc.tile_pool(name="ep", bufs=2) as ep, \
             tc.tile_pool(name="ps1", bufs=4, space="PSUM") as ps1, \
             tc.tile_pool(name="ps2", bufs=2, space="PSUM") as ps2, \
             tc.tile_pool(name="ps3", bufs=1, space="PSUM") as ps3, \
             tc.tile_pool(name="ps4", bufs=1, space="PSUM") as ps4:

            # ---- resident weights: big ones on SWDGE, small on sync ring
            ws_sb = wp.tile([sc_w, n_sc * D1], F32R)
            nc.gpsimd.dma_start(ws_sb[:], ws_d[:])
            wb_sb = wp.tile([P_pad, n_blk * OT], F32R)
            nc.gpsimd.dma_start(wb_sb[:], wb_d[:])
            w2_sb = wp.tile([OT, n_kc2 * H], F32R)
            nc.gpsimd.dma_start(w2_sb[:], w2_d[:])
            w3_sb = wp.tile([H, C], F32R)
            nc.sync.dma_start(w3_sb[:], w3_d[:])
            b1_sb = wp.tile([OT, n_blk], F32)
            nc.sync.dma_start(b1_sb[:], b1_d[:])
            b2_sb = wp.tile([H, 1], F32)
            nc.sync.dma_start(b2_sb[:], b2_d[:])
            b3_sb = wp.tile([C, 1], F32)
            nc.sync.dma_start(b3_sb[:], b3_d[:])
            ones_sb = wp.tile([C, C], F32R)
            nc.sync.dma_start(ones_sb[:], ones_d[:])

            # ---- emit every superchunk's loads up front (slot semaphores
            # throttle); one stripe DMA + one band DMA per superchunk.
            xs_tiles, xb_tiles = [], []
            for s in range(n_sup):
                ss = s * SC
                xs_t = xp.tile([sc_w, n_sc * SC], F32R, name="xs_t", tag="xs")
                nc.gpsimd.dma_start(
                    xs_t[:], xs_d[:, s * n_sc * SC:(s + 1) * n_sc * SC])
                xb_t = xp.tile([P_pad, n_blk * SC], F32R, name="xb_t", tag="xb")
                nc.gpsimd.dma_start(
                    xb_t[:], xb_d[:, s * n_blk * SC:(s + 1) * n_blk * SC])
                xs_tiles.append(xs_t)
                xb_tiles.append(xb_t)

            for s in range(n_sup):
                ss = s * SC
                xs_t, xb_t = xs_tiles[s], xb_tiles[s]

                for h2 in range(n_half):
                    hs = h2 * NB
                    bs = ss + hs
                    # ---- layer 1 ----
                    y1_t = []
                    for j in range(n_blk):
                        p = ps1.tile([OT, NB], F32, tag="l1", name="p1")
                        for c in range(n_sc):
                            kw = nS - c * sc_w if c == n_sc - 1 else sc_w
                            nc.tensor.matmul(
                                p[:], ws_sb[:kw, c * D1 + j * OT:
                                            c * D1 + (j + 1) * OT],
                                xs_t[:kw, c * SC + hs:c * SC + hs + NB],
                                start=(c == 0), stop=False)
                        nc.tensor.matmul(
                            p[:], wb_sb[:R_lens[j], j * OT:(j + 1) * OT],
                            xb_t[:R_lens[j], j * SC + hs:j * SC + hs + NB],
                            start=False, stop=True)
                        h = hp.tile([OT, NB], F32R, name=f"y1_{j}",
                                    tag=f"y1{j}")
                        if j % 2 == 0:
                            nc.vector.tensor_scalar(h[:], p[:],
                                                    b1_sb[:, j:j + 1], 0.0,
                                                    op0=ALU.add, op1=ALU.max)
                        else:
                            nc.scalar.activation(h[:], p[:], AF.Relu,
                                                 bias=b1_sb[:, j:j + 1])
                        y1_t.append(h)

                    # ---- layer 2 ----
                    p2 = ps2.tile([H, NB], F32, tag="l2", name="p2")
                    for k in range(n_kc2):
                        nc.tensor.matmul(p2[:], w2_sb[:, k * H:(k + 1) * H],
                                         y1_t[k][:], start=(k == 0),
                                         stop=(k == n_kc2 - 1))
                    y2 = hp.tile([H, NB], F32R, tag="y2")
                    nc.scalar.activation(y2[:], p2[:], AF.Relu,
                                         bias=b2_sb[:, 0:1])

                    # ---- layer 3 + log_softmax (lse via all-ones matmul:
                    # one MM sums exp() across the 10 class partitions AND
                    # broadcasts the fp32 sum to all of them) ----
                    p3 = ps3.tile([C, NB], F32, tag="l3", name="p3")
                    nc.tensor.matmul(p3[:], w3_sb[:], y2[:], start=True,
                                     stop=True)
                    y3t = hp.tile([C, NB], F32, tag="y3t")
                    nc.scalar.activation(y3t[:], p3[:], AF.Identity,
                                         bias=b3_sb[:, 0:1])
                    ex = hp.tile([C, NB], F32R, tag="ex")
                    nc.scalar.activation(ex[:], y3t[:], AF.Exp)
                    ps_l = ps4.tile([C, NB], F32, tag="lse", name="ps_l")
                    nc.tensor.matmul(ps_l[:], ones_sb[:], ex[:], start=True,
                                     stop=True)
                    ls = hp.tile([C, NB], F32, tag="ls")
                    nc.scalar.activation(ls[:], ps_l[:], AF.Ln)
                    o = ep.tile([C, NB], F32, tag="o")
                    nc.vector.tensor_tensor(o[:], y3t[:], ls[:],
                                            op=ALU.subtract)
                    nc.scalar.dma_start(out_d[:, bs:bs + NB], o[:])

    nc.compile()
    return nc


_CACHE = {}


def _prepare(x, W1, b1, W2, b2, W3, b3, mask1, mask2, mask3):
    B, D1 = x.shape
    H = W2.shape[0]
    C = W3.shape[0]
    assert B % N_CORES == 0
    Bc = B // N_CORES

    S, R_list = _decompose_mask1(np.asarray(mask1))
    R_lens = [len(r) for r in R_list]
    n_blk = len(R_list)
    P_pad = max(R_lens)
    nS = len(S)
    n_sc = (nS + 127) // 128
    sc_w = -(-nS // n_sc)

    Wm1 = (np.asarray(W1) * np.asarray(mask1)).astype(np.float32)
    Wm2 = (np.asarray(W2) * np.asarray(mask2)).astype(np.float32)
    Wm3 = (np.asarray(W3) * np.asarray(mask3)).astype(np.float32)

    # stripe weights packed [sc_w, n_sc*D1]; chunk c in columns [c*D1,(c+1)*D1)
    ws = np.zeros((sc_w, n_sc * D1), np.float32)
    for c in range(n_sc):
        rows = S[c * sc_w:(c + 1) * sc_w]
        ws[:len(rows), c * D1:c * D1 + D1] = Wm1[:, rows].T
    wb = np.zeros((P_pad, n_blk * OT), np.float32)
    for j, R in enumerate(R_list):
        wb[:len(R), j * OT:j * OT + OT] = Wm1[j * OT:(j + 1) * OT, R].T
    n_kc2 = D1 // OT
    w2 = np.ascontiguousarray(
        Wm2.T.reshape(n_kc2, OT, H).transpose(1, 0, 2).reshape(OT, n_kc2 * H))
    w3 = np.ascontiguousarray(Wm3.T)                      # [H, C]
    b1p = np.ascontiguousarray(
        np.asarray(b1, np.float32).reshape(n_blk, OT).T)  # [OT, n_blk]
    b2p = np.asarray(b2, np.float32).reshape(H, 1)
    b3p = np.asarray(b3, np.float32).reshape(C, 1)

    xT = np.asarray(x, np.float32).T                      # [D1, B] view
    n_sup = Bc // SC
    # stripe rows packed [sc_w, n_sc, B] then reordered so each per-core
    # superchunk is one contiguous slab: [sc_w, NC, n_sup, n_sc, SC]
    xs_all = np.zeros((sc_w, n_sc, B), np.float32)
    for c in range(n_sc):
        rows = S[c * sc_w:(c + 1) * sc_w]
        xs_all[:len(rows), c] = xT[rows]
    xs_all = np.ascontiguousarray(
        xs_all.reshape(sc_w, n_sc, N_CORES, n_sup, SC)
              .transpose(0, 2, 3, 1, 4))
    xb_all = np.zeros((P_pad, n_blk, B), np.float32)
    for j, R in enumerate(R_list):
        xb_all[:len(R), j] = xT[R]
    xb_all = np.ascontiguousarray(
        xb_all.reshape(P_pad, n_blk, N_CORES, n_sup, SC)
              .transpose(0, 2, 3, 1, 4))

    if _MM_NP is None:
        import ml_dtypes
        cast = lambda a: np.asarray(a, dtype=ml_dtypes.bfloat16)
    elif _MM_NP is np.float16:
        cast = lambda a: np.asarray(a, dtype=np.float16)
    else:
        cast = lambda a: np.asarray(a, dtype=np.float32)
    xs_all = cast(xs_all); xb_all = cast(xb_all)
    ws = cast(ws); wb = cast(wb); w2 = cast(w2); w3 = cast(w3)
    meta = dict(nS=nS, R_lens=R_lens, P_pad=P_pad, Bc=Bc, D1=D1, H=H, C=C)
    key = (B, D1, H, C, nS, tuple(R_lens), _MM_DT_NAME)
    if key not in _CACHE:
        _CACHE[key] = _build_program(meta)
    nc = _CACHE[key]

    in_maps = []
    for c in range(N_CORES):
        sl = slice(c * Bc, (c + 1) * Bc)
        in_maps.append({
            "xs": xs_all[:, c].reshape(sc_w, n_sc * Bc),
            "xb": xb_all[:, c].reshape(P_pad, n_blk * Bc),
            "ws": ws, "wb": wb, "w2": w2, "w3": w3,
            "b1": b1p, "b2": b2p, "b3": b3p,
            "ones": cast(np.ones((C, C), np.float32)),
        })
    return nc, in_maps, meta


def _assemble(results, meta):
    outs = [np.ascontiguousarray(results[c]["out"].T)     # [Bc, C]
            for c in range(N_CORES)]
    return np.concatenate(outs, axis=0).astype(np.float32)


def kernel(**inputs):
    nc, in_maps, meta = _prepare(**inputs)
    res = bass_utils.run_bass_kernel_spmd(nc, in_maps,
                                          core_ids=list(range(N_CORES)))
    return _assemble(res.results, meta)


def kernel_traced(tmpdir=None, **inputs):
    """Same as kernel() but with NTFF profiling; returns (output, results)."""
    nc, in_maps, meta = _prepare(**inputs)
    res = bass_utils.run_bass_kernel_spmd(nc, in_maps,
                                          core_ids=list(range(N_CORES)),
                                          trace=True, tmpdir=tmpdir)
    return _assemble(res.results, meta), res

